# revision 44
# baseline (speedup 1.0000x reference)
"""Trainium2 Bass kernel for pairwise-GEMM + 8-bin histc + L2 normalize.

Strategy (hardcoded for bs=64, F=256, M=128, BINS=8, 8 cores):
  - scores[a,b] = matf[a]^T @ matf[b]; histogram over all M*M elements is
    invariant under transpose, so only diagonals d=0..32 of (a,b) pairs are
    computed; host mirrors to (b,a).
  - Inputs are cast to fp16 and PRE-SWIZZLED on the host: for each a-unit
    the rhs block is stored c-major (column f = c*32 + d over the 32 pair
    diagonals) so the PE streams a fully packed ifmap (1 cycle/row), PSUM
    chunks land in staging order, and the PSUM->SBUF fp16 casts are packed
    copies on ACT/DVE.
  - Pair-major layout via the XBAR DMA transpose: each a-unit's staging
    tile [128, 4096] is transposed in ONE DMA into a unit tile [128, 4096]
    where partition q holds elements of pair (q mod 32) only.  Stats,
    thresholds and counting then become full-width per-partition ops:
      * min/max: 2 DVE tensor_reduce over 4096 elems
      * per-pair combine + threshold build: PE transposes of the [128,1]
        quarter-stats, one fused J/8 tensor_scalar, a selector matmul
      * counting: 7 thresholds split DVE (is_lt+accum) / ACT (Sign+accum)
  - d=32 pairs of the 16 main a-units form 2 leftover units (8 pairs on
    128 partitions, q mod 8).
  - Software-pipelined emission: glue(u-1), gemm+cast+xbar(u), counts(u-2),
    stats(u) so no engine queue head-blocks on another engine.
  - Host: per-pair sums over partition groups -> cum counts -> bins -> L2
    normalize -> mirror.
"""

import sys
import os

for _p in ("/opt/trn_rl_repo", "/root/.axon_site/_ro/trn_rl_repo"):
    if os.path.isdir(_p) and _p not in sys.path:
        sys.path.insert(0, _p)

import numpy as np

import concourse.bass as bass
import concourse.bacc as bacc
import concourse.mybir as mybir
from concourse import bass_utils
from concourse.tile import TileContext

F32 = mybir.dt.float32
F16 = mybir.dt.float16
AF = mybir.ActivationFunctionType
ALU = mybir.AluOpType
AX = mybir.AxisListType

BS = 64          # batch (a/b index range)
FD = 256         # feature dim (contraction)
M = 128          # matrix dim -> partition dim
BINS = 8
WINA = 4         # a-values per window
NWIN = 2         # windows per core
NMATF = 2
NPAIR_ELEMS = M * M                     # 16384
NMAIN = NMATF * NWIN * WINA             # 16 main units (32 pairs each)
NUNITS = NMAIN + NMATF                  # + 2 leftover units (8 pairs each)
UCOLS = 32 * M + M + M                  # 4096 swizzled + d32 tail + lhs

# Engine assignment for the 7 thresholds (j = 1..7).  DVE counts via
# is_lt+accum, ACT via Sign+accum (host decodes (N-S)/2).  ACT_JS must be
# contiguous.
def _js_for(u):
    if u % 4 == 0:
        return (1, 2), (3, 4, 5, 6, 7)
    return (1, 2, 3), (4, 5, 6, 7)


NOUT = 8  # out cols: DVE cums then ACT signsums, padded

_CACHED = {}


def _core_bases(c):
    """(window0 base a, window1 base a) for core c."""
    return 4 * c, 60 - 4 * c


def _build_nc():
    nc = bacc.Bacc(
        "TRN2", target_bir_lowering=False, debug=False, enable_asserts=False
    )
    # per-unit input [unit, K-half, 128, UCOLS]: swizzled rhs + d32 + lhs
    xu_d = nc.dram_tensor("xu", [NMAIN, 2, M, UCOLS], F16, kind="ExternalInput")
    ident_d = nc.dram_tensor("ident", [M, M], F32, kind="ExternalInput")
    # qsel32[d, q] = 1 iff q % 32 == d ; qsel8[d, q] = 1 iff q % 8 == d
    qsel32_d = nc.dram_tensor("qsel32", [32, M], F32, kind="ExternalInput")
    qsel8_d = nc.dram_tensor("qsel8", [8, M], F32, kind="ExternalInput")
    j8_d = nc.dram_tensor("j8", [32, 7], F32, kind="ExternalInput")
    out_d = nc.dram_tensor(
        "out", [NUNITS, M, NOUT], F32, kind="ExternalOutput"
    )

    with TileContext(nc) as tc:
        with (
            tc.tile_pool(name="xu", bufs=5) as xu_pool,
            tc.tile_pool(name="const", bufs=1) as const_pool,
            tc.tile_pool(name="stg", bufs=4) as stg_pool,
            tc.tile_pool(name="d32", bufs=1) as d32_pool,
            tc.tile_pool(name="unit", bufs=5) as unit_pool,
            tc.tile_pool(name="junk", bufs=1) as junk_pool,
            tc.tile_pool(name="sml", bufs=4) as sml,
            tc.tile_pool(name="cnt", bufs=3) as cnt_pool,
            tc.tile_pool(name="pg", bufs=5, space="PSUM") as psum_g,
            tc.tile_pool(name="pt", bufs=1, space="PSUM") as psum_t,
            tc.tile_pool(name="ptm", bufs=1, space="PSUM") as psum_tm,
            tc.tile_pool(name="pq", bufs=1, space="PSUM") as psum_q,
        ):
            # ---- consts ------------------------------------------------------
            ident = const_pool.tile([M, M], F32, tag="ident")
            nc.sync.dma_start(ident[:], ident_d[:])
            qsel32 = const_pool.tile([32, M], F32, tag="qsel32")
            nc.sync.dma_start(qsel32[:], qsel32_d[:])
            qsel8 = const_pool.tile([8, M], F32, tag="qsel8")
            nc.sync.dma_start(qsel8[:], qsel8_d[:])
            j8 = const_pool.tile([32, 7], F32, tag="j8")
            nc.sync.dma_start(j8[:], j8_d[:])

            junkD = junk_pool.tile([M, 4096], F16, tag="junkD")
            junkA = junk_pool.tile([M, 4096], F16, tag="junkA")

            # d32 staging: per matf, c-major: free idx = c*8 + aidx
            d32stg = [
                d32_pool.tile([M, 8 * M], F16, tag=f"d32_{m}", name=f"d32_{m}")
                for m in range(NMATF)
            ]

            xu_tiles = {}

            def fetch_xu(u, ways=2):
                # split each K-half into `ways` DMAs so they spread across
                # DMA engines (one engine per DMA ~17 GB/s)
                t = xu_pool.tile([M, 2 * UCOLS], F16, tag="xu")
                q = UCOLS // ways
                for k in range(2):
                    for i in range(ways):
                        lo = i * q
                        hi = UCOLS if i == ways - 1 else (i + 1) * q
                        nc.sync.dma_start(
                            t[:, k * UCOLS + lo : k * UCOLS + hi],
                            xu_d[u, k, :, lo:hi],
                        )
                xu_tiles[u] = t

            # ---- pipeline stages --------------------------------------------
            def stage_gemm(u):
                """GEMM + packed cast + XBAR transpose for main unit u."""
                m = u // (NWIN * WINA)
                aidx = u % (NWIN * WINA)
                xut = xu_tiles.pop(u)
                rhs0 = xut[:, 0:UCOLS]
                rhs1 = xut[:, UCOLS : 2 * UCOLS]
                lhs0 = rhs0[:, 33 * M : 34 * M]
                lhs1 = rhs1[:, 33 * M : 34 * M]
                stg = stg_pool.tile([M, 32 * M], F16, tag="stg")

                # 8 chunks of 512 (16 c-values x 32 d), K blocked in pairs
                # of chunks so the stationary operand is reloaded rarely.
                for blk in range(4):
                    banks = []
                    chs = range(2 * blk, 2 * blk + 2)
                    for ch in chs:
                        bank = psum_g.tile([M, 512], F32, tag="bank")
                        nc.tensor.matmul(
                            bank[:], lhs0, rhs0[:, 512 * ch : 512 * (ch + 1)],
                            start=True, stop=False,
                        )
                        banks.append(bank)
                    for i, ch in enumerate(chs):
                        nc.tensor.matmul(
                            banks[i], lhs1, rhs1[:, 512 * ch : 512 * (ch + 1)],
                            start=False, stop=True,
                        )
                    for i, ch in enumerate(chs):
                        dst = stg[:, 512 * ch : 512 * (ch + 1)]
                        nc.scalar.copy(dst, banks[i][:])
                # d=32 pair -> d32 staging (free idx c*8 + aidx)
                bank = psum_g.tile([M, 512], F32, tag="bank")
                nc.tensor.matmul(
                    bank[:, 0:M], lhs0, rhs0[:, 32 * M : 33 * M],
                    start=True, stop=False,
                )
                nc.tensor.matmul(
                    bank[:, 0:M], lhs1, rhs1[:, 32 * M : 33 * M],
                    start=False, stop=True,
                )
                dst = d32stg[m][:].rearrange("p (c a) -> p a c", a=8)[
                    :, aidx : aidx + 1, :
                ]
                nc.scalar.copy(
                    dst, bank[:, 0:M].rearrange("p (x c) -> p x c", x=1)
                )

                ut = unit_pool.tile([M, 32 * M], F16, tag="ut")
                h = 16 * M
                nc.sync.dma_start(
                    ut[:, 0:h].rearrange("q (k p) -> q k p", p=M),
                    stg[:, 0:h], transpose=True,
                )
                nc.sync.dma_start(
                    ut[:, h : 2 * h].rearrange("q (k p) -> q k p", p=M),
                    stg[:, h : 2 * h], transpose=True,
                )
                return {"ut": ut, "u": u, "npair": 32}

            def stage_gemm_leftover(m):
                ut = unit_pool.tile([M, 8 * M], F16, tag="utL", name="utL")
                nc.sync.dma_start(
                    ut[:].rearrange("q (k p) -> q k p", p=M),
                    d32stg[m][:], transpose=True,
                )
                return {"ut": ut, "u": NMAIN + m, "npair": 8}

            def stage_stats(st):
                ut = st["ut"]
                w = ut.shape[1]
                h = w // 2
                pmm = sml.tile([M, 2], F32, tag="pmm")
                if w > 8 * M:
                    p4 = sml.tile([M, 4], F32, tag="p4")
                    nc.vector.tensor_reduce(
                        p4[:, 0:1], ut[:, 0:h], AX.X, ALU.min
                    )
                    nc.vector.tensor_reduce(
                        p4[:, 1:2], ut[:, 0:h], AX.X, ALU.max
                    )
                    nc.vector.tensor_reduce(
                        p4[:, 2:3], ut[:, h:w], AX.X, ALU.min
                    )
                    nc.vector.tensor_reduce(
                        p4[:, 3:4], ut[:, h:w], AX.X, ALU.max
                    )
                    nc.vector.tensor_tensor(
                        pmm[:, 0:1], p4[:, 0:1], p4[:, 2:3], ALU.min
                    )
                    nc.vector.tensor_tensor(
                        pmm[:, 1:2], p4[:, 1:2], p4[:, 3:4], ALU.max
                    )
                else:
                    nc.vector.tensor_reduce(pmm[:, 0:1], ut[:], AX.X, ALU.min)
                    nc.vector.tensor_reduce(pmm[:, 1:2], ut[:], AX.X, ALU.max)
                st["pmm"] = pmm

            def stage_glue(st):
                """Per-pair stats combine + threshold build + broadcast."""
                npair = st["npair"]
                glueT = psum_t.tile([1, 2 * M], F32, tag="glueT")
                nc.tensor.transpose(glueT[:, 0:M], st["pmm"][:, 0:1], ident[:])
                nc.tensor.transpose(
                    glueT[:, M : 2 * M], st["pmm"][:, 1:2], ident[:]
                )
                # q = npair*r + d  ->  view [1, npair(d), nrep(r)]
                mn = sml.tile([1, npair], F32, tag="mn")
                mx = sml.tile([1, npair], F32, tag="mx")
                gmin = glueT[:, 0:M].rearrange("o (r d) -> o d r", d=npair)
                gmax = glueT[:, M : 2 * M].rearrange(
                    "o (r d) -> o d r", d=npair
                )
                nc.vector.tensor_reduce(mn[:], gmin, AX.X, ALU.min)
                nc.vector.tensor_reduce(mx[:], gmax, AX.X, ALU.max)
                # back onto npair partitions
                mmT = psum_tm.tile([32, 2], F32, tag="mmT")
                nc.tensor.transpose(
                    mmT[0:npair, 0:1], mn[:], ident[0:1, 0:1]
                )
                nc.tensor.transpose(
                    mmT[0:npair, 1:2], mx[:], ident[0:1, 0:1]
                )
                mms = sml.tile([32, 2], F32, tag="mms")
                nc.vector.tensor_copy(mms[0:npair, :], mmT[0:npair, :])
                deltaT = sml.tile([32, 1], F32, tag="deltaT")
                nc.vector.tensor_tensor(
                    deltaT[0:npair, :], mms[0:npair, 1:2],
                    mms[0:npair, 0:1], ALU.subtract,
                )
                # thr_j = (j/8)*delta + mn
                thrP = sml.tile([32, 7], F32, tag="thrP")
                for j in range(1, 8):
                    nc.vector.tensor_scalar(
                        thrP[0:npair, j - 1 : j],
                        deltaT[0:npair, :], float(j) / BINS,
                        mms[0:npair, 0:1], ALU.mult, ALU.add,
                    )
                qsel = qsel32 if npair == 32 else qsel8
                thrqP = psum_q.tile([M, 8], F32, tag="thrqP")
                nc.tensor.matmul(thrqP[:, 0:7], qsel[:], thrP[0:npair, :])
                thrq = sml.tile([M, 7], F32, tag="thrq")
                nc.vector.tensor_copy(thrq[:], thrqP[:, 0:7])
                aj0 = _js_for(st["u"])[1][0]
                nthrq = sml.tile([M, 5], F32, tag="nthrq")
                nc.vector.tensor_scalar(
                    nthrq[:, 0 : 8 - aj0], thrq[:, aj0 - 1 : 7],
                    -1.0, None, ALU.mult,
                )
                st["thrq"], st["nthrq"] = thrq, nthrq

            def stage_count(st):
                ut = st["ut"]
                u = st["u"]
                DVE_JS, ACT_JS = _js_for(u)
                thrq, nthrq = st["thrq"], st["nthrq"]
                cntD = cnt_pool.tile([M, 4], F32, tag="cntD")
                cntA = cnt_pool.tile([M, 5], F32, tag="cntA")
                for i, j in enumerate(DVE_JS):
                    nc.vector.tensor_scalar(
                        junkD[:, 0 : ut.shape[1]], ut[:],
                        thrq[:, j - 1 : j], None,
                        ALU.is_lt, ALU.add, accum_out=cntD[:, i : i + 1],
                    )
                for i, j in enumerate(ACT_JS):
                    nc.scalar.activation(
                        junkA[:, 0 : ut.shape[1]], ut[:], AF.Sign,
                        bias=nthrq[:, i : i + 1],
                        accum_out=cntA[:, i : i + 1],
                    )
                nD, nA = len(DVE_JS), len(ACT_JS)
                nc.scalar.dma_start(out_d[u, :, 0:nD], cntD[:, 0:nD])
                nc.scalar.dma_start(
                    out_d[u, :, nD : nD + nA], cntA[:, 0:nA]
                )

            # ---- run the pipeline -------------------------------------------
            states = []
            # stagger startup: first two units get low-latency 8-way
            # fetches with no competing prefetch traffic; depth-4 prefetch
            # catches up over iterations 0-1.
            fetch_xu(0, ways=8)
            fetch_xu(1, ways=8)
            for i in range(NUNITS):
                if i >= 1:
                    stage_glue(states[i - 1])
                if i == 0 or i == 1:
                    fetch_xu(2 * i + 2)
                    fetch_xu(2 * i + 3)
                elif i + 4 < NMAIN:
                    fetch_xu(i + 4)
                if i < NMAIN:
                    st = stage_gemm(i)
                else:
                    st = stage_gemm_leftover(i - NMAIN)
                states.append(st)
                if i >= 2:
                    stage_count(states[i - 2])
                stage_stats(st)
            stage_glue(states[NUNITS - 1])
            stage_count(states[NUNITS - 2])
            stage_count(states[NUNITS - 1])
    nc.compile()
    return nc


def _host_consts():
    ident = np.eye(M, dtype=np.float32)
    q = np.arange(M)
    qsel32 = (q[None, :] % 32 == np.arange(32)[:, None]).astype(np.float32)
    qsel8 = (q[None, :] % 8 == np.arange(8)[:, None]).astype(np.float32)
    j8 = np.tile(
        (np.arange(1, 8, dtype=np.float32) / BINS)[None, :], (32, 1)
    )
    return ident, qsel32, qsel8, j8


def profile_exec_ns(tmpdir="/tmp/bass_hist_trace"):
    """Re-run the last kernel() invocation with NTFF tracing; returns ns."""
    if "nc" not in _CACHED or "in_maps" not in _CACHED:
        return None
    import shutil

    shutil.rmtree(tmpdir, ignore_errors=True)
    os.makedirs(tmpdir, exist_ok=True)
    res = bass_utils.run_bass_kernel_spmd(
        _CACHED["nc"], _CACHED["in_maps"], list(range(8)),
        trace=True, tmpdir=tmpdir,
    )
    return res.exec_time_ns


def kernel(matf1: np.ndarray, matf2: np.ndarray) -> np.ndarray:
    if "nc" not in _CACHED:
        _CACHED["nc"] = _build_nc()
    nc = _CACHED["nc"]
    ident, qsel32, qsel8, j8 = _host_consts()

    # X[m]: [2 K-halves, 128, 64*128] fp16
    Xs = []
    for matf in (matf1, matf2):
        X = np.ascontiguousarray(
            np.asarray(matf, dtype=np.float32).transpose(1, 0, 2)
        ).reshape(2, M, BS * M).astype(np.float16)
        Xs.append(X)

    # per-(m, a) swizzled rhs blocks [2, 128, UCOLS]:
    # cols f = c*32 + d -> source col ((a+d)%64)*128 + c, plus d=32 tail
    c_idx = np.arange(M)
    d_idx = np.arange(32)
    E = np.empty((NMATF, BS, 2, M, UCOLS), dtype=np.float16)
    for m in range(NMATF):
        for a in range(BS):
            cols = (((a + d_idx[None, :]) % BS) * M + c_idx[:, None]).reshape(
                -1
            )  # [128*32] in (c, d) order
            tail = ((a + 32) % BS) * M + c_idx
            lhsc = a * M + c_idx
            idx = np.concatenate([cols, tail, lhsc])
            E[m, a] = Xs[m][:, :, idx]

    in_maps = []
    for c in range(8):
        b0, b1 = _core_bases(c)
        xu = np.empty((NMAIN, 2, M, UCOLS), dtype=np.float16)
        for m in range(NMATF):
            for w, base in enumerate((b0, b1)):
                for al in range(WINA):
                    u = (m * NWIN + w) * WINA + al
                    a = (base + al) % BS
                    xu[u] = E[m, a]
        in_maps.append(
            {"xu": xu, "ident": ident, "qsel32": qsel32,
             "qsel8": qsel8, "j8": j8}
        )

    _CACHED["in_maps"] = in_maps
    res = bass_utils.run_bass_kernel_spmd(nc, in_maps, list(range(8)))

    N = float(NPAIR_ELEMS)

    def cols_for(u):
        dj, aj = _js_for(u)
        return (list(dj) + list(aj),
                [False] * len(dj) + [True] * len(aj))

    cums = np.zeros((NMATF, BS, BS, 7), dtype=np.float64)
    for c in range(8):
        out = np.asarray(res.results[c]["out"], dtype=np.float64)
        b0, b1 = _core_bases(c)
        for m in range(NMATF):
            # main units: 32 pairs, pair(q) = q % 32
            for w, base in enumerate((b0, b1)):
                for al in range(WINA):
                    u = (m * NWIN + w) * WINA + al
                    a = (base + al) % BS
                    col_js, col_is_sign = cols_for(u)
                    per_pair = out[u].reshape(4, 32, NOUT).sum(axis=0)
                    for ci, j in enumerate(col_js):
                        v = per_pair[:, ci]
                        if col_is_sign[ci]:
                            v = np.floor((N - v) / 2)
                        for d in range(32):
                            b = (a + d) % BS
                            cums[m, a, b, j - 1] = v[d]
            # leftover unit: 8 pairs (d=32), pair(q) = q % 8
            uL = NMAIN + m
            col_js, col_is_sign = cols_for(uL)
            per_pair = out[uL].reshape(16, 8, NOUT).sum(axis=0)
            for ci, j in enumerate(col_js):
                v = per_pair[:, ci]
                if col_is_sign[ci]:
                    v = np.floor((N - v) / 2)
                for aidx in range(8):
                    w, al = divmod(aidx, WINA)
                    a = ((b0, b1)[w] + al) % BS
                    b = (a + 32) % BS
                    cums[m, a, b, j - 1] = v[aidx]

    # mirror to (b, a)
    for m in range(NMATF):
        for a in range(BS):
            for d in range(33):
                b = (a + d) % BS
                cums[m, b, a] = cums[m, a, b]

    counts = np.zeros((NMATF, BS, BS, BINS), dtype=np.float64)
    counts[..., 0] = cums[..., 0]
    counts[..., 1:7] = np.diff(cums, axis=-1)
    counts[..., 7] = N - cums[..., 6]
    counts = counts.astype(np.float32).reshape(NMATF, BS * BS, BINS)
    norm = np.linalg.norm(counts, axis=-1, keepdims=True)
    h = counts / np.maximum(norm, 1e-12)
    return np.concatenate([h[0], h[1]], axis=-1)
